# revision 4
# baseline (speedup 1.0000x reference)
"""Trainium2 Bass kernel for nn_MultiHeadAttention_72765335929540.

Reference semantics (B=8, S=2048, D=512, H=8 identical heads, d_k=d_v=64):
    q = query @ Wq + bq;  k = key @ Wk + bk;  v = key @ Wv + bv   (bug: v from key)
    scores = q k^T / 8 (+ causal mask if training);  att = softmax(scores)
    head = att @ v;  out = tile(head, 8) @ Wo + bo = head @ Wo_eff + bo
where Wo_eff = sum_h Wo[64h:64h+64].  `value` is never read.

Distribution: data-parallel, one batch element per NeuronCore (8 cores).

Per-core pipeline (bf16 compute, f32 accumulate in PSUM):
  1. f32 natural loads of query/key via HWDGE (2KB packets), issued upfront
     (sync: query, scalar: key), one DMA per 512-row group
  2. f32 -> bf16 casts on DVE (query) / GpSimd (key)
  3. X^T via DMA XBAR transpose (sync HWDGE) into block layout
     xT[p, b, cc, j] = X[b*128+j, cc*128+p]   -- zero PE cycles
  4. qT = Wq^T Xq^T (+bq on gpsimd eviction); kT|vT packed = [Wk|Wv]^T Xk^T
     (+bias on DVE eviction); v' blocks via XBAR transpose of kvT rows 64:127
     + ones column (for the softmax denominator)
  5. per key-block J: scoresT[j,i] = kT_J^T qT (PE), pT = exp(scoresT/8) (ACT,
     no max-subtraction -- scores are provably < ~3), causal diag mask via
     trineg matmul accumulation
  6. headT'[d,i] (d<64: sum_j v pT; d=64: denominator l_i) accumulated on PE
  7. out_b = (headT'^T @ [Wo_eff; bo]) * (1/l_i) -- normalization + bias fused,
     final muls on DVE, one output store per 512-row sweep (sync)

PSUM budget (8 banks): sc x4 (proj psums + scoresT pieces), ha x1 (headT'
accumulator), po x2 (final out psum), pl x1 (l-column transposes).
"""
import sys

sys.path.insert(0, "/opt/trn_rl_repo")

import numpy as np
import ml_dtypes

import concourse.bass as bass
import concourse.mybir as mybir
import concourse.tile as tile
from concourse.bass_utils import run_bass_kernel_spmd

BF = mybir.dt.bfloat16
F32 = mybir.dt.float32
S, D, DK = 2048, 512, 64
NB = S // 128          # 16 blocks of 128
H = 8

# ---------------------------------------------------------------------------
# walrus workaround: this build's ISA structs hold few semaphore waits per
# instruction; split the excess onto same-engine NoOps (1 wait each).
_ws_counter = [0]
_CTRL_TYPES = ("InstDrain", "InstNoOp", "InstEventSemaphore", "InstBranch")


def _split_sync_waits(nc, max_waits=1, max_updates=2):
    for f in nc.m.functions:
        for blk in f.blocks:
            insts = blk.instructions
            i = 0
            while i < len(insts):
                inst = insts[i]
                si = inst.sync_info
                if si is None:
                    i += 1
                    continue
                ctrl = type(inst).__name__ in _CTRL_TYPES
                max_w = 1 if ctrl else max_waits
                max_u = 1 if ctrl else max_updates
                waits = list(si.on_wait)
                updates = list(si.on_update)
                if len(waits) <= max_w and len(updates) <= max_u:
                    i += 1
                    continue
                keep_w = waits[-max_w:] if len(waits) > max_w else waits
                extra_w = waits[:-max_w] if len(waits) > max_w else []
                keep_u = updates[:max_u] if len(updates) > max_u else updates
                extra_u = updates[max_u:] if len(updates) > max_u else []
                inst.sync_info = mybir.SyncInfo(on_wait=keep_w, on_update=keep_u)
                pre, post = [], []
                for w in extra_w:
                    _ws_counter[0] += 1
                    nop = mybir.InstNoOp(name=f"WSPLIT-{_ws_counter[0]}", ins=[], outs=[])
                    nop.engine = inst.engine
                    nop.sync_info = mybir.SyncInfo(on_wait=[w], on_update=[])
                    pre.append(nop)
                for u in extra_u:
                    _ws_counter[0] += 1
                    nop = mybir.InstNoOp(name=f"USPLIT-{_ws_counter[0]}", ins=[], outs=[])
                    nop.engine = inst.engine
                    nop.sync_info = mybir.SyncInfo(on_wait=[], on_update=[u])
                    post.append(nop)
                for k, nop in enumerate(pre):
                    insts.insert(i + k, nop)
                for k, nop in enumerate(post):
                    insts.insert(i + len(pre) + 1 + k, nop)
                i += len(pre) + 1 + len(post)


# ---------------------------------------------------------------------------
def _build_nc(masked: bool):
    nc = bass.Bass()
    q_d = nc.declare_dram_parameter("query", [S, D], F32, isOutput=False)
    k_d = nc.declare_dram_parameter("key", [S, D], F32, isOutput=False)
    wq_d = nc.declare_dram_parameter("wq", [D, DK], BF, isOutput=False)
    wkv_d = nc.declare_dram_parameter("wkv", [D, 128], BF, isOutput=False)
    bq_d = nc.declare_dram_parameter("bq", [DK, 1], F32, isOutput=False)
    bkv_d = nc.declare_dram_parameter("bkv", [128, 1], F32, isOutput=False)
    frhs_d = nc.declare_dram_parameter("frhs", [DK + 1, D], BF, isOutput=False)
    trineg_d = nc.declare_dram_parameter("trineg", [128, 128], BF, isOutput=False)
    id_d = nc.declare_dram_parameter("ident", [128, 128], BF, isOutput=False)
    out_d = nc.declare_dram_parameter("out", [S, D], F32, isOutput=True)

    Exp = mybir.ActivationFunctionType.Exp

    with tile.TileContext(nc) as tc:
        with (
            tc.tile_pool(name="pers", bufs=1) as pers,
            tc.tile_pool(name="natq", bufs=4) as natq,
            tc.tile_pool(name="natk", bufs=4) as natk,
            tc.tile_pool(name="cb", bufs=2) as cb,
            tc.tile_pool(name="hts", bufs=3) as hts,
            tc.tile_pool(name="osb", bufs=2) as osb,
            tc.tile_pool(name="ps", bufs=2, space="PSUM") as ps,
        ):
            # ---- constants (sync issues q-loads first; consts after) -------
            # query group loads: f32 natural, [128, 4, 512]; rows g*128+p.
            qn = [natq.tile([128, 4, D], F32, tag="qn", name=f"qn{g}")
                  for g in range(4)]
            for g in range(4):
                nc.sync.dma_start(
                    qn[g][:],
                    q_d[g * 512:(g + 1) * 512, :].rearrange(
                        "(gg p) d -> p gg d", p=128))
            kn = [natk.tile([128, 4, D], F32, tag="kn", name=f"kn{g}")
                  for g in range(4)]
            for g in range(4):
                nc.scalar.dma_start(
                    kn[g][:],
                    k_d[g * 512:(g + 1) * 512, :].rearrange(
                        "(gg p) d -> p gg d", p=128))

            wq_sb = pers.tile([128, 4, DK], BF, tag="wq")
            nc.sync.dma_start(wq_sb[:], wq_d[:].rearrange("(c p) k -> p c k", p=128))
            wkv_sb = pers.tile([128, 4, 128], BF, tag="wkv")
            nc.sync.dma_start(wkv_sb[:], wkv_d[:].rearrange("(c p) k -> p c k", p=128))
            bq_sb = pers.tile([DK, 1], F32, tag="bq")
            nc.sync.dma_start(bq_sb[:], bq_d[:])
            bkv_sb = pers.tile([128, 1], F32, tag="bkv")
            nc.sync.dma_start(bkv_sb[:], bkv_d[:])
            id_sb = pers.tile([128, 128], BF, tag="id")
            nc.sync.dma_start(id_sb[:], id_d[:])
            frhs_sb = pers.tile([DK + 1, D], BF, tag="frhs")
            nc.scalar.dma_start(frhs_sb[:], frhs_d[:])
            trineg_sb = pers.tile([128, 128], BF, tag="trineg")
            nc.scalar.dma_start(trineg_sb[:], trineg_d[:])

            # persistent activations
            # block layout: xT[p, b, cc, j] = X[b*128 + j, cc*128 + p]
            xqT = pers.tile([128, NB, 4, 128], BF, tag="xqT")
            xkT = pers.tile([128, NB, 4, 128], BF, tag="xkT")
            qT = pers.tile([DK, S], BF, tag="qT")
            kvT = pers.tile([128, S], BF, tag="kvT")
            # v' blocks: vp_all[p, b, f] = v[b*128+p, f] for f<64; col 64 = 1.
            # XBAR needs a contiguous dest, so v lands in vp_nat first and
            # gpsimd copies it into the 65-pitch layout.
            vp_nat = pers.tile([128, NB, DK], BF, tag="vpn")
            vp_all = pers.tile([128, NB, DK + 1], BF, tag="vp")

            # pT storage for the whole causal band
            Ws = [(S - 128 * J) if masked else S for J in range(NB)]
            pts = {}

            # ---- per-group staging ----------------------------------------
            def stage_q(g):
                qb = cb.tile([128, 4, D], BF, tag="qb", name=f"qb{g}")
                nc.vector.tensor_copy(qb[:], qn[g][:])
                # XBAR: dest [128, 16, 128] (4 b-blocks x 4 cc), contiguous
                nc.sync.dma_start_transpose(
                    xqT[:, 4 * g:4 * g + 4, :, :], qb[:])
                sl = slice(g * 512, (g + 1) * 512)
                pq = ps.tile([DK, 512], F32, tag="sc", name=f"pq_{g}", bufs=4)
                for cc in range(4):
                    nc.tensor.matmul(pq[:],
                                     lhsT=wq_sb[:, cc, :],
                                     rhs=xqT[:, 4 * g:4 * g + 4, cc, :],
                                     start=(cc == 0), stop=(cc == 3))
                nc.vector.tensor_scalar_add(qT[:, sl], pq[:], bq_sb[:, 0:1])

            def stage_k(g):
                kb = cb.tile([128, 4, D], BF, tag="kb", name=f"kb{g}")
                nc.gpsimd.tensor_copy(kb[:], kn[g][:])
                nc.sync.dma_start_transpose(
                    xkT[:, 4 * g:4 * g + 4, :, :], kb[:])
                sl = slice(g * 512, (g + 1) * 512)
                pkv = ps.tile([128, 512], F32, tag="sc", name=f"pkv_{g}", bufs=4)
                for cc in range(4):
                    nc.tensor.matmul(pkv[:],
                                     lhsT=wkv_sb[:, cc, :],
                                     rhs=xkT[:, 4 * g:4 * g + 4, cc, :],
                                     start=(cc == 0), stop=(cc == 3))
                nc.vector.tensor_scalar_add(kvT[:, sl], pkv[:], bkv_sb[:, 0:1])
                # v' for the 4 j-blocks of this group: XBAR of kvT rows 64:128
                nc.sync.dma_start_transpose(
                    vp_nat[:, 4 * g:4 * g + 4, :], kvT[64:128, sl])
                nc.gpsimd.tensor_copy(vp_all[:, 4 * g:4 * g + 4, 0:DK],
                                      vp_nat[:, 4 * g:4 * g + 4, :])
                nc.gpsimd.memset(vp_all[:, 4 * g:4 * g + 4, DK:DK + 1], 1.0)

            # ---- finalize one 512-row sweep -------------------------------
            def finalize_tile(t, ht4):
                """ht4 = evicted [65, 512] headT' of blocks 4t..4t+3."""
                ot = osb.tile([128, 4, D], F32, tag="ot", name=f"ot_{t}")
                for b in range(4 * t, 4 * t + 4):
                    c0 = (b % 4) * 128
                    pl = ps.tile([128, 1], BF, tag="pl", name=f"pl_{b}", bufs=1)
                    nc.tensor.transpose(pl[:], ht4[DK:DK + 1, c0:c0 + 128],
                                        id_sb[64:65, 64:65])
                    r = hts.tile([128, 1], F32, tag="r", name=f"r_{b}")
                    nc.vector.reciprocal(r[:], pl[:, 0:1])
                    po = ps.tile([128, 512], F32, tag="po", name=f"po_{b}", bufs=2)
                    nc.tensor.matmul(po[:], lhsT=ht4[:, c0:c0 + 128], rhs=frhs_sb[:],
                                     start=True, stop=True)
                    nc.vector.tensor_scalar_mul(ot[:, b % 4, :], po[:], r[:, 0:1])
                # store from scalar: sync's queue must stay free for the
                # next group's XBAR transposes
                nc.scalar.dma_start(
                    out_d[t * 512:(t + 1) * 512, :].rearrange(
                        "(gg p) d -> p gg d", p=128),
                    ot[:])

            # ---- sweeps over query pieces ---------------------------------
            # staging g is emitted one sweep ahead so sweep p-1's compute
            # overlaps group p's cast/XBAR/projection
            stage_q(0)
            stage_k(0)
            stage_q(1)
            stage_k(1)
            for p in range(4):
                if p + 2 < 4:
                    stage_q(p + 2)
                    stage_k(p + 2)
                Jmax = 4 * p + 3 if masked else NB - 1
                # scores pieces (J, p) + exp
                for J in range(0, Jmax + 1):
                    if J not in pts:
                        pts[J] = pers.tile([128, Ws[J]], BF, tag=f"pt{J}",
                                           name=f"pt_{J}")
                    pt = pts[J]
                    i_start = max(512 * p, 128 * J) if masked else 512 * p
                    w = 512 * p + 512 - i_start
                    x0 = i_start - (128 * J if masked else 0)
                    psc = ps.tile([128, 512], F32, tag="sc", name=f"sc_{J}_{p}",
                                  bufs=4)
                    diag = masked and J // 4 == p
                    nc.tensor.matmul(psc[:, 0:w],
                                     lhsT=kvT[0:DK, J * 128:(J + 1) * 128],
                                     rhs=qT[:, i_start:i_start + w],
                                     start=True, stop=not diag,
                                     skip_group_check=True)
                    if diag:
                        # accumulate -1e30 upper-triangle into the diag block
                        nc.tensor.matmul(psc[:, 0:128], lhsT=id_sb[:],
                                         rhs=trineg_sb[:], start=False, stop=True,
                                         skip_group_check=True)
                    nc.scalar.activation(pt[:, x0:x0 + w], psc[:, 0:w],
                                         Exp, scale=0.125)
                # head-tile p: one psum accumulation over all J
                hacc = ps.tile([DK + 1, 512], F32, tag="ha", name=f"ha_{p}", bufs=1)
                for J in range(0, Jmax + 1):
                    b_lo = max(4 * p, J) if masked else 4 * p
                    wdt = (4 * p + 4 - b_lo) * 128
                    c0 = (b_lo % 4) * 128
                    x = (128 * (b_lo - J) if masked else 512 * p)
                    nc.tensor.matmul(hacc[:, c0:c0 + wdt],
                                     lhsT=vp_all[:, J, :], rhs=pts[J][:, x:x + wdt],
                                     start=(J == 0), stop=(J == Jmax),
                                     skip_group_check=True)
                ht4 = hts.tile([DK + 1, 512], BF, tag="ht", name=f"ht4_{p}")
                nc.vector.tensor_copy(ht4[:], hacc[:])
                finalize_tile(p, ht4)

    _split_sync_waits(nc)
    return nc


_NC_CACHE = {}


def _get_nc(masked: bool):
    if masked not in _NC_CACHE:
        _NC_CACHE[masked] = _build_nc(masked)
    return _NC_CACHE[masked]


# ---------------------------------------------------------------------------
def kernel(query, key, value, Wq, bq, Wk, bk, Wv, bv, Wo, bo, training):
    query = np.asarray(query, dtype=np.float32)
    key = np.asarray(key, dtype=np.float32)
    Wq = np.asarray(Wq, dtype=np.float64)
    Wk = np.asarray(Wk, dtype=np.float64)
    Wv = np.asarray(Wv, dtype=np.float64)
    Wo = np.asarray(Wo, dtype=np.float64)
    bq_h = np.asarray(bq, dtype=np.float32).reshape(DK, 1)
    bk_h = np.asarray(bk, dtype=np.float32).reshape(DK, 1)
    bv_h = np.asarray(bv, dtype=np.float32).reshape(DK, 1)
    bo_h = np.asarray(bo, dtype=np.float64)
    masked = bool(np.asarray(training).item())

    B = query.shape[0]
    wq_h = Wq.astype(ml_dtypes.bfloat16)
    wkv_h = np.concatenate([Wk, Wv], axis=1).astype(ml_dtypes.bfloat16)
    bkv_h = np.concatenate([bk_h, bv_h], axis=0)
    wo_eff = Wo.reshape(H, DK, D).sum(axis=0)
    frhs_h = np.concatenate([wo_eff, bo_h[None, :]], axis=0).astype(ml_dtypes.bfloat16)
    jj, ii = np.meshgrid(np.arange(128), np.arange(128), indexing="ij")
    trineg_h = np.where(jj <= ii, 0.0, -1e30).astype(ml_dtypes.bfloat16)
    id_h = np.eye(128, dtype=ml_dtypes.bfloat16)

    consts = {"wq": wq_h, "wkv": wkv_h, "bq": bq_h, "bkv": bkv_h,
              "frhs": frhs_h, "trineg": trineg_h, "ident": id_h}
    in_maps = [dict(consts, query=np.ascontiguousarray(query[i]),
                    key=np.ascontiguousarray(key[i])) for i in range(B)]
    global _last_in_maps
    _last_in_maps = in_maps

    nc = _get_nc(masked)
    res = run_bass_kernel_spmd(nc, in_maps, core_ids=list(range(B)))
    return np.stack([np.asarray(res.results[i]["out"], dtype=np.float32)
                     for i in range(B)])
